# revision 43
# baseline (speedup 1.0000x reference)
"""Multi-head causal attention (B=2, S=2048, E=1024, H=16, D=64) on 8 trn2 cores.

Sharding (Megatron-style, per hint): data-parallel over batch (2) x
tensor-parallel over heads (4 groups of 4 heads / 256 features).
Core c: batch c//4, head-group c%4.

Per-core device program (SPMD, identical on all cores), pipelined over 4
query chunks of 512:
  A(ic). PE-transpose x rows -> xT (bf16) for the chunk's 4 i-tiles
  B(ic). qT/kT projections in [n, s] layout; v in natural [s, n] layout (bf16)
  C(ic). causal attention in transposed-score layout:
       sT[j,i] = kT_h . qT_h (K=64 matmul), p = exp(s/8) on ScalarE (bf16),
       causal mask via multiplicative 0/1 tiles on diagonal blocks,
       ctxT[d,i] accumulated with v-augmented-with-ones stationary ->
       row 64 of psum = softmax denominator; normalize with
       copy + partition_broadcast + tensor_tensor divide
  D(ic). AllGather the normalized ctxT chunk across the 4-core batch group
  E(ic). out[:, g*256:(g+1)*256] = ctxT_full.T @ Wo[:, slice] + bo[slice]
       (E is emitted one chunk behind so the collective hides behind compute)
Host only slices inputs and concatenates the 8 disjoint output slices.
"""

import contextlib

import ml_dtypes
import numpy as np

import concourse.mybir as mybir
import concourse.tile as tile
from concourse import bacc
from concourse.bass_utils import run_bass_kernel_spmd

F32 = mybir.dt.float32
BF16 = mybir.dt.bfloat16
F32R = mybir.dt.float32r
import os
MMDT = {"bf16": BF16, "f32r": F32R}[os.environ.get("MM_DT", "f32r")]

B, S, E, H, D = 2, 2048, 1024, 16, 64
N_CORES = 8
TP = 4                 # tensor-parallel degree (head groups per batch)
NSL = E // TP          # 256 features per core
HLOC = H // TP         # 4 heads per core
KT = E // 128          # 8 contraction tiles
IT = S // 128          # 16 sequence tiles
ICH = S // 512         # 4 sequence chunks of 512
SCALE = 1.0 / np.sqrt(D)

REPLICA_GROUPS = [[0, 1, 2, 3], [4, 5, 6, 7]]

_cache: dict = {}


def _emit(nc, tc, prm):
    x, wq, bq, wk, bk, wv, bv, wo, bo, ident, masks, out = prm

    with contextlib.ExitStack() as stack:
        ent = stack.enter_context
        const = ent(tc.tile_pool(name="const", bufs=1))
        wstage = ent(tc.tile_pool(name="wstage", bufs=2))
        wpool = ent(tc.tile_pool(name="wpool", bufs=1))
        xrow_p = ent(tc.tile_pool(name="xrow", bufs=2))
        xt_p = ent(tc.tile_pool(name="xt", bufs=2))
        qkv_p = ent(tc.tile_pool(name="qkv", bufs=1))
        psum_t = ent(tc.tile_pool(name="psum_t", bufs=1, space="PSUM"))
        psum_mm = ent(tc.tile_pool(name="psum_mm", bufs=2, space="PSUM"))
        psum_s = ent(tc.tile_pool(name="psum_s", bufs=2, space="PSUM"))
        psum_c = ent(tc.tile_pool(name="psum_c", bufs=3, space="PSUM"))
        pwork = ent(tc.tile_pool(name="pwork", bufs=4))
        norm_p = ent(tc.tile_pool(name="norm", bufs=2))
        ctxt_p = ent(tc.tile_pool(name="ctxt", bufs=1))
        ctxf_p = ent(tc.tile_pool(name="ctxf", bufs=1))
        osb_p = ent(tc.tile_pool(name="osb", bufs=2))
        dram = ent(tc.tile_pool(name="dram", bufs=1, space="DRAM"))

        # ---- constants ----
        ident_sb = const.tile([128, 128], F32R)
        nc.sync.dma_start(out=ident_sb[:], in_=ident[:].bitcast(F32R))
        emitted = {}
        warm_in = dram.tile([1, 128], F32)
        warm_out = dram.tile([TP, 1, 128], F32)
        nc.gpsimd.collective_compute(
            "AllGather", mybir.AluOpType.bypass,
            replica_groups=REPLICA_GROUPS,
            ins=[warm_in.opt()], outs=[warm_out.opt()],
        )
        mask_sb = const.tile([128, HLOC, 512], BF16)

        # ---- weights (dtype MMDT; f32r loads directly, bf16 converts) ----
        wq_sb = wpool.tile([128, KT, NSL], MMDT)
        wk_sb = wpool.tile([128, KT, NSL], MMDT)
        wv_sb = wpool.tile([128, KT, NSL], MMDT)
        wo_sb = wpool.tile([128, KT, NSL], MMDT)
        for w_sb, w_dr in ((wq_sb, wq), (wk_sb, wk), (wv_sb, wv), (wo_sb, wo)):
            if MMDT == F32R:
                w_r = w_dr.rearrange("(t p) n -> p t n", p=128).bitcast(F32R)
                for kt in range(KT):
                    nc.gpsimd.dma_start(
                        out=w_sb[:, kt, :], in_=w_r[:, kt, :])
            else:
                wst = wstage.tile([128, KT, NSL], F32, tag="wst")
                nc.gpsimd.dma_start(
                    out=wst[:], in_=w_dr.rearrange("(t p) n -> p t n", p=128))
                nc.vector.tensor_copy(w_sb[:], wst[:])
        nc.gpsimd.dma_start(out=mask_sb[:], in_=masks[:])
        bq_sb = wpool.tile([128, 2], F32)
        bk_sb = wpool.tile([128, 2], F32)
        for b_sb, b_dr in ((bq_sb, bq), (bk_sb, bk)):
            nc.sync.dma_start(out=b_sb[:], in_=b_dr.rearrange("(t p) -> p t", p=128))
        bv_row = wpool.tile([1, NSL], F32)
        nc.sync.dma_start(out=bv_row[:], in_=bv[None, :])
        bvb = wpool.tile([128, NSL], F32)
        nc.gpsimd.partition_broadcast(out_ap=bvb[:], in_ap=bv_row[:])
        bo_row = wpool.tile([1, NSL], F32)
        nc.sync.dma_start(out=bo_row[:], in_=bo[None, :])
        bob = wpool.tile([128, NSL], F32)
        nc.gpsimd.partition_broadcast(out_ap=bob[:], in_ap=bo_row[:])

        # ---- persistent activations ----
        qt_sb = qkv_p.tile([128, 2, S], MMDT)
        kt_sb = qkv_p.tile([128, 2, S], MMDT)
        v_sb = qkv_p.tile([128, IT, HLOC, D + 1], MMDT)
        ones_col = qkv_p.tile([128, IT, HLOC, 1], F32)
        nc.vector.memset(ones_col[:], 1.0)
        nc.vector.tensor_copy(v_sb[:, :, :, D:D + 1], ones_col[:])
        ctxt_sb = ctxt_p.tile([128, 2, S], MMDT)

        # DRAM bounce buffers for the chunked allgather (distinct per chunk
        # so chunk ic+1's send never waits on chunk ic's collective)
        cc_in = [dram.tile([NSL, 512], MMDT, name=f"cc_in{ic}") for ic in range(ICH)]
        cc_out = [dram.tile([TP, NSL, 512], MMDT, name=f"cc_out{ic}")
                  for ic in range(ICH)]

        def stage_a_alloc(ic):
            # per-chunk xT tile + x row loads; transposes come as parts
            xt_sb = xt_p.tile([128, KT, 512], MMDT, tag="xt")
            xrs = []
            for k4, it in enumerate(range(4 * ic, 4 * ic + 4)):
                xr = xrow_p.tile([128, E], F32R, tag="xr", bufs=8)
                nc.sync.dma_start(
                    out=xr[:], in_=x[it * 128:(it + 1) * 128, :].bitcast(F32R))
                xrs.append(xr)
            return xt_sb, xrs

        def stage_a_part(xt_sb, xrs, part):
            kt = part
            pt = psum_t.tile([128, 512], F32, tag="pt")
            for k4 in range(4):
                nc.tensor.transpose(
                    pt[:, k4 * 128:(k4 + 1) * 128].bitcast(F32R),
                    xrs[k4][:, kt * 128:(kt + 1) * 128],
                    ident_sb[:])
            nc.scalar.copy(out=xt_sb[:, kt, :], in_=pt[:])

        def stage_a(ic):
            xt_sb, xrs = stage_a_alloc(ic)
            for part in range(KT):
                stage_a_part(xt_sb, xrs, part)
            return xt_sb

        def stage_b(ic, xt_sb):
            for (w_sb, b_sb, o_sb, osl) in (
                    (wq_sb, bq_sb, qt_sb, slice(ic * 512, (ic + 1) * 512)),
                    (wk_sb, bk_sb, kt_sb, slice(ic * 512, (ic + 1) * 512))):
                for nt in range(2):
                    pm = psum_mm.tile([128, 512], F32, tag="pqk")
                    for kt in range(KT):
                        nc.tensor.matmul(
                            pm[:],
                            w_sb[:, kt, nt * 128:(nt + 1) * 128],
                            xt_sb[:, kt, :],
                            start=(kt == 0), stop=(kt == KT - 1),
                        )
                    nc.vector.tensor_scalar_add(
                        out=o_sb[:, nt, osl],
                        in0=pm[:], scalar1=b_sb[:, nt:nt + 1])
            for k4, it in enumerate(range(4 * ic, 4 * ic + 4)):
                pv = psum_mm.tile([128, NSL], F32, tag="pqk")
                for kt in range(KT):
                    nc.tensor.matmul(
                        pv[:],
                        xt_sb[:, kt, k4 * 128:(k4 + 1) * 128],
                        wv_sb[:, kt, :],
                        start=(kt == 0), stop=(kt == KT - 1),
                    )
                nc.vector.tensor_add(
                    out=v_sb[:, it, :, 0:D],
                    in0=pv[:].rearrange("p (h d) -> p h d", d=D),
                    in1=bvb[:].rearrange("p (h d) -> p h d", d=D))

        def stage_c(ic, fillers=None):
            i0 = ic * 512
            njt = 4 * (ic + 1)

            def emit_s(h, jt):
                # diagonal j-tiles only need columns i_local >= 128*dt_
                nt, base = divmod(h, 2)
                base *= D
                dt_ = jt - 4 * ic
                c0 = max(dt_, 0) * 128
                ps = psum_s.tile([128, 512], F32, tag="ps")
                nc.tensor.matmul(
                    ps[:, c0:],
                    kt_sb[base:base + D, nt, jt * 128:(jt + 1) * 128],
                    qt_sb[base:base + D, nt, i0 + c0:i0 + 512],
                    start=True, stop=True,
                )
                pw = pwork.tile([128, 512], MMDT, tag="pw")
                nc.scalar.activation(
                    out=pw[:, c0:], in_=ps[:, c0:],
                    func=mybir.ActivationFunctionType.Exp, scale=float(SCALE))
                if dt_ >= 0:
                    nc.vector.tensor_mul(
                        pw[:, c0:c0 + 128], pw[:, c0:c0 + 128],
                        mask_sb[:, 0, 0:128])
                return pw

            def emit_ctx(h, jt, pc, pw):
                c0 = max(jt - 4 * ic, 0) * 128
                nc.tensor.matmul(
                    pc[:, c0:],
                    v_sb[:, jt, h, :],
                    pw[:, c0:],
                    start=(jt == 0), stop=(jt == njt - 1),
                )

            for h in range(HLOC):
                nt, base = divmod(h, 2)
                base *= D
                pc = psum_c.tile([D + 1, 512], F32, tag="pc")
                pw_prev = emit_s(h, 0)
                for jt in range(1, njt):
                    pw = emit_s(h, jt)
                    emit_ctx(h, jt - 1, pc, pw_prev)
                    pw_prev = pw
                emit_ctx(h, njt - 1, pc, pw_prev)
                lrow = norm_p.tile([1, 512], F32, tag="lrow")
                nc.vector.reciprocal(out=lrow[:], in_=pc[D:D + 1, :])
                lb = norm_p.tile([D, 512], F32, tag="lb")
                nc.gpsimd.partition_broadcast(out_ap=lb[:], in_ap=lrow[:])
                nc.vector.tensor_mul(
                    ctxt_sb[base:base + D, nt, i0:i0 + 512],
                    pc[0:D, :], lb[:])
                for f in (fillers or {}).get(h, []):
                    f()

        def stage_d(ic):
            i0 = ic * 512
            nc.sync.dma_start(
                out=cc_in[ic].rearrange("(t p) i -> p t i", p=128),
                in_=ctxt_sb[:, :, i0:i0 + 512])
            nc.gpsimd.collective_compute(
                "AllGather", mybir.AluOpType.bypass,
                replica_groups=REPLICA_GROUPS,
                ins=[cc_in[ic].opt()], outs=[cc_out[ic].opt()],
            )

        cc_in_h = [dram.tile([NSL, 256], MMDT, name=f"cc_in_h{k}") for k in range(2)]
        cc_out_h = [dram.tile([TP, NSL, 256], MMDT, name=f"cc_out_h{k}")
                    for k in range(2)]

        def stage_d_half(ic, half):
            i0 = ic * 512 + half * 256
            nc.sync.dma_start(
                out=cc_in_h[half].rearrange("(t p) i -> p t i", p=128),
                in_=ctxt_sb[:, :, i0:i0 + 256])
            nc.gpsimd.collective_compute(
                "AllGather", mybir.AluOpType.bypass,
                replica_groups=REPLICA_GROUPS,
                ins=[cc_in_h[half].opt()], outs=[cc_out_h[half].opt()],
            )

        def stage_e_half(ic, half):
            ctxf_sb = ctxf_p.tile([128, KT, 256], MMDT, tag="ctxf_h", bufs=2)
            nc.sync.dma_start(
                out=ctxf_sb[:],
                in_=cc_out_h[half].rearrange("g (t p) i -> p (g t) i", p=128))
            for k in range(2):
                it = 4 * ic + 2 * half + k
                po = psum_mm.tile([128, NSL], F32, tag="pqk")
                for ct in range(KT):
                    nc.tensor.matmul(
                        po[:],
                        ctxf_sb[:, ct, k * 128:(k + 1) * 128],
                        wo_sb[:, ct, :],
                        start=(ct == 0), stop=(ct == KT - 1),
                    )
                ot = osb_p.tile([128, NSL], F32, tag="ot")
                nc.vector.tensor_add(out=ot[:], in0=po[:], in1=bob[:])
                nc.gpsimd.dma_start(out=out[it * 128:(it + 1) * 128, :], in_=ot[:])

        def stage_e_load(ic):
            ctxf_sb = ctxf_p.tile([128, KT, 512], MMDT, tag="ctxf")
            nc.sync.dma_start(
                out=ctxf_sb[:],
                in_=cc_out[ic].rearrange("g (t p) i -> p (g t) i", p=128))
            return ctxf_sb

        def stage_e_part(ic, ctxf_sb, k):
            it = 4 * ic + k
            po = psum_mm.tile([128, NSL], F32, tag="pqk")
            for ct in range(KT):
                nc.tensor.matmul(
                    po[:],
                    ctxf_sb[:, ct, k * 128:(k + 1) * 128],
                    wo_sb[:, ct, :],
                    start=(ct == 0), stop=(ct == KT - 1),
                )
            ot = osb_p.tile([128, NSL], F32, tag="ot")
            nc.vector.tensor_add(out=ot[:], in0=po[:], in1=bob[:])
            nc.gpsimd.dma_start(out=out[it * 128:(it + 1) * 128, :], in_=ot[:])

        def stage_e(ic):
            ctxf_sb = stage_e_load(ic)
            for k in range(4):
                stage_e_part(ic, ctxf_sb, k)

        # pipeline: A(ic+1) transpose parts and E(ic-1) out-proj groups are
        # interleaved between C(ic) attention heads to keep PE dense; the
        # chunked allgather(ic) hides behind C(ic+1).
        emitted = None
        xt_cur = stage_a(0)
        for ic in range(ICH):
            stage_b(ic, xt_cur)
            fillers = {h: [] for h in range(HLOC)}
            if ic + 1 < ICH:
                xt_next, xrs_next = stage_a_alloc(ic + 1)
                for h in range(HLOC):
                    k = 2 * h
                    fillers[h].append(
                        lambda xt=xt_next, xr=xrs_next, kk=k:
                            stage_a_part(xt, xr, kk))
                    fillers[h].append(
                        lambda xt=xt_next, xr=xrs_next, kk=k + 1:
                            stage_a_part(xt, xr, kk))
            else:
                xt_next = None
            if ic > 0:
                ctxf_prev = stage_e_load(ic - 1)
                for h in range(HLOC):
                    fillers[h].append(
                        lambda icc=ic - 1, cf=ctxf_prev, kk=h:
                            stage_e_part(icc, cf, kk))
            stage_c(ic, fillers)
            stage_d(ic)
            xt_cur = xt_next
        stage_e(ICH - 1)


# revision 44
# speedup vs baseline: 1.1035x; 1.1035x over previous
"""Multi-head causal attention (B=2, S=2048, E=1024, H=16, D=64) on 8 trn2 cores.

Sharding (Megatron-style, per hint): data-parallel over batch (2) x
tensor-parallel over heads (4 groups of 4 heads / 256 features).
Core c: batch c//4, head-group c%4.

Per-core device program (SPMD, identical on all cores), pipelined over 4
query chunks of 512:
  A(ic). PE-transpose x rows -> xT (bf16) for the chunk's 4 i-tiles
  B(ic). qT/kT projections in [n, s] layout; v in natural [s, n] layout (bf16)
  C(ic). causal attention in transposed-score layout:
       sT[j,i] = kT_h . qT_h (K=64 matmul), p = exp(s/8) on ScalarE (bf16),
       causal mask via multiplicative 0/1 tiles on diagonal blocks,
       ctxT[d,i] accumulated with v-augmented-with-ones stationary ->
       row 64 of psum = softmax denominator; normalize with
       copy + partition_broadcast + tensor_tensor divide
  D(ic). AllGather the normalized ctxT chunk across the 4-core batch group
  E(ic). out[:, g*256:(g+1)*256] = ctxT_full.T @ Wo[:, slice] + bo[slice]
       (E is emitted one chunk behind so the collective hides behind compute)
Host only slices inputs and concatenates the 8 disjoint output slices.
"""

import contextlib

import ml_dtypes
import numpy as np

import concourse.mybir as mybir
import concourse.tile as tile
from concourse import bacc
from concourse.bass_utils import run_bass_kernel_spmd

F32 = mybir.dt.float32
BF16 = mybir.dt.bfloat16
F32R = mybir.dt.float32r
import os
MMDT = {"bf16": BF16, "f32r": F32R}[os.environ.get("MM_DT", "f32r")]

B, S, E, H, D = 2, 2048, 1024, 16, 64
N_CORES = 8
TP = 4                 # tensor-parallel degree (head groups per batch)
NSL = E // TP          # 256 features per core
HLOC = H // TP         # 4 heads per core
KT = E // 128          # 8 contraction tiles
IT = S // 128          # 16 sequence tiles
ICH = S // 512         # 4 sequence chunks of 512
SCALE = 1.0 / np.sqrt(D)

REPLICA_GROUPS = [[0, 1, 2, 3], [4, 5, 6, 7]]

_cache: dict = {}


def _emit(nc, tc, prm):
    x, wq, bq, wk, bk, wv, bv, wo, bo, ident, masks, out = prm

    with contextlib.ExitStack() as stack:
        ent = stack.enter_context
        const = ent(tc.tile_pool(name="const", bufs=1))
        wstage = ent(tc.tile_pool(name="wstage", bufs=2))
        wpool = ent(tc.tile_pool(name="wpool", bufs=1))
        xrow_p = ent(tc.tile_pool(name="xrow", bufs=2))
        xt_p = ent(tc.tile_pool(name="xt", bufs=2))
        qkv_p = ent(tc.tile_pool(name="qkv", bufs=1))
        psum_t = ent(tc.tile_pool(name="psum_t", bufs=2, space="PSUM"))
        psum_mm = ent(tc.tile_pool(name="psum_mm", bufs=2, space="PSUM"))
        psum_s = ent(tc.tile_pool(name="psum_s", bufs=2, space="PSUM"))
        psum_c = ent(tc.tile_pool(name="psum_c", bufs=2, space="PSUM"))
        pwork = ent(tc.tile_pool(name="pwork", bufs=4))
        norm_p = ent(tc.tile_pool(name="norm", bufs=2))
        ctxt_p = ent(tc.tile_pool(name="ctxt", bufs=1))
        ctxf_p = ent(tc.tile_pool(name="ctxf", bufs=1))
        osb_p = ent(tc.tile_pool(name="osb", bufs=2))
        dram = ent(tc.tile_pool(name="dram", bufs=1, space="DRAM"))

        # ---- constants ----
        ident_sb = const.tile([128, 128], F32R)
        nc.sync.dma_start(out=ident_sb[:], in_=ident[:].bitcast(F32R))
        emitted = {}
        warm_in = dram.tile([1, 128], F32)
        warm_out = dram.tile([TP, 1, 128], F32)
        nc.gpsimd.collective_compute(
            "AllGather", mybir.AluOpType.bypass,
            replica_groups=REPLICA_GROUPS,
            ins=[warm_in.opt()], outs=[warm_out.opt()],
        )
        mask_sb = const.tile([128, HLOC, 512], BF16)

        # ---- weights (dtype MMDT; f32r loads directly, bf16 converts) ----
        wq_sb = wpool.tile([128, KT, NSL], MMDT)
        wk_sb = wpool.tile([128, KT, NSL], MMDT)
        wv_sb = wpool.tile([128, KT, NSL], MMDT)
        wo_sb = wpool.tile([128, KT, NSL], MMDT)
        for w_sb, w_dr in ((wq_sb, wq), (wk_sb, wk), (wv_sb, wv), (wo_sb, wo)):
            if MMDT == F32R:
                w_r = w_dr.rearrange("(t p) n -> p t n", p=128).bitcast(F32R)
                for kt in range(KT):
                    nc.gpsimd.dma_start(
                        out=w_sb[:, kt, :], in_=w_r[:, kt, :])
            else:
                wst = wstage.tile([128, KT, NSL], F32, tag="wst")
                nc.gpsimd.dma_start(
                    out=wst[:], in_=w_dr.rearrange("(t p) n -> p t n", p=128))
                nc.vector.tensor_copy(w_sb[:], wst[:])
        nc.gpsimd.dma_start(out=mask_sb[:], in_=masks[:])
        bq_sb = wpool.tile([128, 2], F32)
        bk_sb = wpool.tile([128, 2], F32)
        for b_sb, b_dr in ((bq_sb, bq), (bk_sb, bk)):
            nc.sync.dma_start(out=b_sb[:], in_=b_dr.rearrange("(t p) -> p t", p=128))
        bv_row = wpool.tile([1, NSL], F32)
        nc.sync.dma_start(out=bv_row[:], in_=bv[None, :])
        bvb = wpool.tile([128, NSL], F32)
        nc.gpsimd.partition_broadcast(out_ap=bvb[:], in_ap=bv_row[:])
        bo_row = wpool.tile([1, NSL], F32)
        nc.sync.dma_start(out=bo_row[:], in_=bo[None, :])
        bob = wpool.tile([128, NSL], F32)
        nc.gpsimd.partition_broadcast(out_ap=bob[:], in_ap=bo_row[:])

        # ---- persistent activations ----
        qt_sb = qkv_p.tile([128, 2, S], MMDT)
        kt_sb = qkv_p.tile([128, 2, S], MMDT)
        v_sb = qkv_p.tile([128, IT, HLOC, D + 1], MMDT)
        ones_col = qkv_p.tile([128, IT, HLOC, 1], F32)
        nc.vector.memset(ones_col[:], 1.0)
        nc.vector.tensor_copy(v_sb[:, :, :, D:D + 1], ones_col[:])
        ctxt_sb = ctxt_p.tile([128, 2, S], MMDT)

        # DRAM bounce buffers for the chunked allgather (distinct per chunk
        # so chunk ic+1's send never waits on chunk ic's collective)
        cc_in = [dram.tile([NSL, 512], MMDT, name=f"cc_in{ic}") for ic in range(ICH)]
        cc_out = [dram.tile([TP, NSL, 512], MMDT, name=f"cc_out{ic}")
                  for ic in range(ICH)]

        def stage_a_alloc(ic):
            # per-chunk xT tile + x row loads; transposes come as parts
            xt_sb = xt_p.tile([128, KT, 512], MMDT, tag="xt")
            xrs = []
            for k4, it in enumerate(range(4 * ic, 4 * ic + 4)):
                xr = xrow_p.tile([128, E], F32R, tag="xr", bufs=8)
                nc.sync.dma_start(
                    out=xr[:], in_=x[it * 128:(it + 1) * 128, :].bitcast(F32R))
                xrs.append(xr)
            return xt_sb, xrs

        def stage_a_part(xt_sb, xrs, part):
            kt = part
            pt = psum_t.tile([128, 512], F32, tag="pt")
            for k4 in range(4):
                nc.tensor.transpose(
                    pt[:, k4 * 128:(k4 + 1) * 128].bitcast(F32R),
                    xrs[k4][:, kt * 128:(kt + 1) * 128],
                    ident_sb[:])
            nc.scalar.copy(out=xt_sb[:, kt, :], in_=pt[:])

        def stage_a(ic):
            xt_sb, xrs = stage_a_alloc(ic)
            for part in range(KT):
                stage_a_part(xt_sb, xrs, part)
            return xt_sb

        def stage_b(ic, xt_sb):
            for (w_sb, b_sb, o_sb, osl) in (
                    (wq_sb, bq_sb, qt_sb, slice(ic * 512, (ic + 1) * 512)),
                    (wk_sb, bk_sb, kt_sb, slice(ic * 512, (ic + 1) * 512))):
                for nt in range(2):
                    pm = psum_mm.tile([128, 512], F32, tag="pqk")
                    for kt in range(KT):
                        nc.tensor.matmul(
                            pm[:],
                            w_sb[:, kt, nt * 128:(nt + 1) * 128],
                            xt_sb[:, kt, :],
                            start=(kt == 0), stop=(kt == KT - 1),
                        )
                    nc.vector.tensor_scalar_add(
                        out=o_sb[:, nt, osl],
                        in0=pm[:], scalar1=b_sb[:, nt:nt + 1])
            for k4, it in enumerate(range(4 * ic, 4 * ic + 4)):
                pv = psum_mm.tile([128, NSL], F32, tag="pqk")
                for kt in range(KT):
                    nc.tensor.matmul(
                        pv[:],
                        xt_sb[:, kt, k4 * 128:(k4 + 1) * 128],
                        wv_sb[:, kt, :],
                        start=(kt == 0), stop=(kt == KT - 1),
                    )
                nc.vector.tensor_add(
                    out=v_sb[:, it, :, 0:D],
                    in0=pv[:].rearrange("p (h d) -> p h d", d=D),
                    in1=bvb[:].rearrange("p (h d) -> p h d", d=D))

        def stage_c(ic, fillers=None):
            i0 = ic * 512
            njt = 4 * (ic + 1)

            def emit_s(h, jt):
                # diagonal j-tiles only need columns i_local >= 128*dt_
                nt, base = divmod(h, 2)
                base *= D
                dt_ = jt - 4 * ic
                c0 = max(dt_, 0) * 128
                ps = psum_s.tile([128, 512], F32, tag="ps")
                nc.tensor.matmul(
                    ps[:, c0:],
                    kt_sb[base:base + D, nt, jt * 128:(jt + 1) * 128],
                    qt_sb[base:base + D, nt, i0 + c0:i0 + 512],
                    start=True, stop=True,
                )
                pw = pwork.tile([128, 512], MMDT, tag="pw")
                nc.scalar.activation(
                    out=pw[:, c0:], in_=ps[:, c0:],
                    func=mybir.ActivationFunctionType.Exp, scale=float(SCALE))
                if dt_ >= 0:
                    nc.vector.tensor_mul(
                        pw[:, c0:c0 + 128], pw[:, c0:c0 + 128],
                        mask_sb[:, 0, 0:128])
                return pw

            def emit_ctx(h, jt, pc, pw):
                c0 = max(jt - 4 * ic, 0) * 128
                nc.tensor.matmul(
                    pc[:, c0:],
                    v_sb[:, jt, h, :],
                    pw[:, c0:],
                    start=(jt == 0), stop=(jt == njt - 1),
                )

            for h in range(HLOC):
                nt, base = divmod(h, 2)
                base *= D
                pc = psum_c.tile([D + 1, 512], F32, tag="pc")
                pw_prev = emit_s(h, 0)
                for jt in range(1, njt):
                    pw = emit_s(h, jt)
                    emit_ctx(h, jt - 1, pc, pw_prev)
                    pw_prev = pw
                emit_ctx(h, njt - 1, pc, pw_prev)
                lrow = norm_p.tile([1, 512], F32, tag="lrow")
                nc.vector.reciprocal(out=lrow[:], in_=pc[D:D + 1, :])
                lb = norm_p.tile([D, 512], F32, tag="lb")
                nc.gpsimd.partition_broadcast(out_ap=lb[:], in_ap=lrow[:])
                nc.vector.tensor_mul(
                    ctxt_sb[base:base + D, nt, i0:i0 + 512],
                    pc[0:D, :], lb[:])
                for f in (fillers or {}).get(h, []):
                    f()

        def stage_d(ic):
            i0 = ic * 512
            nc.sync.dma_start(
                out=cc_in[ic].rearrange("(t p) i -> p t i", p=128),
                in_=ctxt_sb[:, :, i0:i0 + 512])
            nc.gpsimd.collective_compute(
                "AllGather", mybir.AluOpType.bypass,
                replica_groups=REPLICA_GROUPS,
                ins=[cc_in[ic].opt()], outs=[cc_out[ic].opt()],
            )

        cc_in_h = [dram.tile([NSL, 256], MMDT, name=f"cc_in_h{k}") for k in range(2)]
        cc_out_h = [dram.tile([TP, NSL, 256], MMDT, name=f"cc_out_h{k}")
                    for k in range(2)]

        def stage_d_half(ic, half):
            i0 = ic * 512 + half * 256
            nc.sync.dma_start(
                out=cc_in_h[half].rearrange("(t p) i -> p t i", p=128),
                in_=ctxt_sb[:, :, i0:i0 + 256])
            nc.gpsimd.collective_compute(
                "AllGather", mybir.AluOpType.bypass,
                replica_groups=REPLICA_GROUPS,
                ins=[cc_in_h[half].opt()], outs=[cc_out_h[half].opt()],
            )

        def stage_e_half(ic, half):
            ctxf_sb = ctxf_p.tile([128, KT, 256], MMDT, tag="ctxf_h", bufs=2)
            nc.sync.dma_start(
                out=ctxf_sb[:],
                in_=cc_out_h[half].rearrange("g (t p) i -> p (g t) i", p=128))
            for k in range(2):
                it = 4 * ic + 2 * half + k
                po = psum_mm.tile([128, NSL], F32, tag="pqk")
                for ct in range(KT):
                    nc.tensor.matmul(
                        po[:],
                        ctxf_sb[:, ct, k * 128:(k + 1) * 128],
                        wo_sb[:, ct, :],
                        start=(ct == 0), stop=(ct == KT - 1),
                    )
                ot = osb_p.tile([128, NSL], F32, tag="ot")
                nc.vector.tensor_add(out=ot[:], in0=po[:], in1=bob[:])
                nc.gpsimd.dma_start(out=out[it * 128:(it + 1) * 128, :], in_=ot[:])

        def stage_e_load(ic):
            ctxf_sb = ctxf_p.tile([128, KT, 512], MMDT, tag="ctxf")
            nc.sync.dma_start(
                out=ctxf_sb[:],
                in_=cc_out[ic].rearrange("g (t p) i -> p (g t) i", p=128))
            return ctxf_sb

        def stage_e_part(ic, ctxf_sb, k):
            it = 4 * ic + k
            po = psum_mm.tile([128, NSL], F32, tag="pqk")
            for ct in range(KT):
                nc.tensor.matmul(
                    po[:],
                    ctxf_sb[:, ct, k * 128:(k + 1) * 128],
                    wo_sb[:, ct, :],
                    start=(ct == 0), stop=(ct == KT - 1),
                )
            ot = osb_p.tile([128, NSL], F32, tag="ot")
            nc.vector.tensor_add(out=ot[:], in0=po[:], in1=bob[:])
            nc.gpsimd.dma_start(out=out[it * 128:(it + 1) * 128, :], in_=ot[:])

        def stage_e(ic):
            ctxf_sb = stage_e_load(ic)
            for k in range(4):
                stage_e_part(ic, ctxf_sb, k)

        # pipeline: A(ic+1) transpose parts and E(ic-1) out-proj groups are
        # interleaved between C(ic) attention heads to keep PE dense; the
        # chunked allgather(ic) hides behind C(ic+1).
        emitted = None
        xt_cur = stage_a(0)
        for ic in range(ICH):
            stage_b(ic, xt_cur)
            fillers = {h: [] for h in range(HLOC)}
            if ic + 1 < ICH:
                xt_next, xrs_next = stage_a_alloc(ic + 1)
                for h in range(HLOC):
                    k = 2 * h
                    fillers[h].append(
                        lambda xt=xt_next, xr=xrs_next, kk=k:
                            stage_a_part(xt, xr, kk))
                    fillers[h].append(
                        lambda xt=xt_next, xr=xrs_next, kk=k + 1:
                            stage_a_part(xt, xr, kk))
            else:
                xt_next = None
            if ic > 0:
                ctxf_prev = stage_e_load(ic - 1)
                for h in range(HLOC):
                    fillers[h].append(
                        lambda icc=ic - 1, cf=ctxf_prev, kk=h:
                            stage_e_part(icc, cf, kk))
            stage_c(ic, fillers)
            stage_d(ic)
            xt_cur = xt_next
        stage_e(ICH - 1)


# revision 46
# speedup vs baseline: 1.1057x; 1.0019x over previous
"""Multi-head causal attention (B=2, S=2048, E=1024, H=16, D=64) on 8 trn2 cores.

Sharding (Megatron-style, per hint): data-parallel over batch (2) x
tensor-parallel over heads (4 groups of 4 heads / 256 features).
Core c: batch c//4, head-group c%4.

Per-core device program (SPMD, identical on all cores), pipelined over 4
query chunks of 512:
  A(ic). PE-transpose x rows -> xT (bf16) for the chunk's 4 i-tiles
  B(ic). qT/kT projections in [n, s] layout; v in natural [s, n] layout (bf16)
  C(ic). causal attention in transposed-score layout:
       sT[j,i] = kT_h . qT_h (K=64 matmul), p = exp(s/8) on ScalarE (bf16),
       causal mask via multiplicative 0/1 tiles on diagonal blocks,
       ctxT[d,i] accumulated with v-augmented-with-ones stationary ->
       row 64 of psum = softmax denominator; normalize with
       copy + partition_broadcast + tensor_tensor divide
  D(ic). AllGather the normalized ctxT chunk across the 4-core batch group
  E(ic). out[:, g*256:(g+1)*256] = ctxT_full.T @ Wo[:, slice] + bo[slice]
       (E is emitted one chunk behind so the collective hides behind compute)
Host only slices inputs and concatenates the 8 disjoint output slices.
"""

import contextlib

import ml_dtypes
import numpy as np

import concourse.mybir as mybir
import concourse.tile as tile
from concourse import bacc
from concourse.bass_utils import run_bass_kernel_spmd

F32 = mybir.dt.float32
BF16 = mybir.dt.bfloat16
F32R = mybir.dt.float32r
import os
MMDT = {"bf16": BF16, "f32r": F32R}[os.environ.get("MM_DT", "f32r")]

B, S, E, H, D = 2, 2048, 1024, 16, 64
N_CORES = 8
TP = 4                 # tensor-parallel degree (head groups per batch)
NSL = E // TP          # 256 features per core
HLOC = H // TP         # 4 heads per core
KT = E // 128          # 8 contraction tiles
IT = S // 128          # 16 sequence tiles
ICH = S // 512         # 4 sequence chunks of 512
SCALE = 1.0 / np.sqrt(D)

REPLICA_GROUPS = [[0, 1, 2, 3], [4, 5, 6, 7]]

_cache: dict = {}


def _emit(nc, tc, prm):
    x, wq, bq, wk, bk, wv, bv, wo, bo, ident, masks, out = prm

    with contextlib.ExitStack() as stack:
        ent = stack.enter_context
        const = ent(tc.tile_pool(name="const", bufs=1))
        wstage = ent(tc.tile_pool(name="wstage", bufs=2))
        wpool = ent(tc.tile_pool(name="wpool", bufs=1))
        xrow_p = ent(tc.tile_pool(name="xrow", bufs=2))
        xt_p = ent(tc.tile_pool(name="xt", bufs=2))
        qkv_p = ent(tc.tile_pool(name="qkv", bufs=1))
        psum_t = ent(tc.tile_pool(name="psum_t", bufs=2, space="PSUM"))
        psum_mm = ent(tc.tile_pool(name="psum_mm", bufs=2, space="PSUM"))
        psum_s = ent(tc.tile_pool(name="psum_s", bufs=2, space="PSUM"))
        psum_c = ent(tc.tile_pool(name="psum_c", bufs=2, space="PSUM"))
        pwork = ent(tc.tile_pool(name="pwork", bufs=4))
        norm_p = ent(tc.tile_pool(name="norm", bufs=2))
        ctxt_p = ent(tc.tile_pool(name="ctxt", bufs=1))
        ctxf_p = ent(tc.tile_pool(name="ctxf", bufs=1))
        osb_p = ent(tc.tile_pool(name="osb", bufs=2))
        dram = ent(tc.tile_pool(name="dram", bufs=1, space="DRAM"))

        # ---- constants ----
        ident_sb = const.tile([128, 128], F32R)
        nc.sync.dma_start(out=ident_sb[:], in_=ident[:].bitcast(F32R))
        emitted = {}
        warm_in = dram.tile([1, 128], F32)
        warm_out = dram.tile([TP, 1, 128], F32)
        nc.gpsimd.collective_compute(
            "AllGather", mybir.AluOpType.bypass,
            replica_groups=REPLICA_GROUPS,
            ins=[warm_in.opt()], outs=[warm_out.opt()],
        )
        mask_sb = const.tile([128, HLOC, 512], BF16)

        # ---- weights (dtype MMDT; f32r loads directly, bf16 converts) ----
        wq_sb = wpool.tile([128, KT, NSL], MMDT)
        wk_sb = wpool.tile([128, KT, NSL], MMDT)
        wv_sb = wpool.tile([128, KT, NSL], MMDT)
        wo_sb = wpool.tile([128, KT, NSL], MMDT)
        for w_sb, w_dr in ((wq_sb, wq), (wk_sb, wk), (wv_sb, wv), (wo_sb, wo)):
            if MMDT == F32R:
                w_r = w_dr.rearrange("(t p) n -> p t n", p=128).bitcast(F32R)
                for kt in range(KT):
                    nc.gpsimd.dma_start(
                        out=w_sb[:, kt, :], in_=w_r[:, kt, :])
            else:
                wst = wstage.tile([128, KT, NSL], F32, tag="wst")
                nc.gpsimd.dma_start(
                    out=wst[:], in_=w_dr.rearrange("(t p) n -> p t n", p=128))
                nc.vector.tensor_copy(w_sb[:], wst[:])
        nc.gpsimd.dma_start(out=mask_sb[:], in_=masks[:])
        bq_sb = wpool.tile([128, 2], F32)
        bk_sb = wpool.tile([128, 2], F32)
        for b_sb, b_dr in ((bq_sb, bq), (bk_sb, bk)):
            nc.sync.dma_start(out=b_sb[:], in_=b_dr.rearrange("(t p) -> p t", p=128))
        bv_row = wpool.tile([1, NSL], F32)
        nc.sync.dma_start(out=bv_row[:], in_=bv[None, :])
        bvb = wpool.tile([128, NSL], F32)
        nc.gpsimd.partition_broadcast(out_ap=bvb[:], in_ap=bv_row[:])
        bo_row = wpool.tile([1, NSL], F32)
        nc.sync.dma_start(out=bo_row[:], in_=bo[None, :])
        bob = wpool.tile([128, NSL], F32)
        nc.gpsimd.partition_broadcast(out_ap=bob[:], in_ap=bo_row[:])

        # ---- persistent activations ----
        qt_sb = qkv_p.tile([128, 2, S], MMDT)
        kt_sb = qkv_p.tile([128, 2, S], MMDT)
        v_sb = qkv_p.tile([128, IT, HLOC, D + 1], MMDT)
        ones_col = qkv_p.tile([128, IT, HLOC, 1], F32)
        nc.vector.memset(ones_col[:], 1.0)
        nc.vector.tensor_copy(v_sb[:, :, :, D:D + 1], ones_col[:])
        ctxt_sb = ctxt_p.tile([128, 2, S], MMDT)

        # DRAM bounce buffers for the chunked allgather (distinct per chunk
        # so chunk ic+1's send never waits on chunk ic's collective)
        cc_in = [dram.tile([NSL, 512], MMDT, name=f"cc_in{ic}") for ic in range(ICH)]
        cc_out = [dram.tile([TP, NSL, 512], MMDT, name=f"cc_out{ic}")
                  for ic in range(ICH)]

        def stage_a_alloc(ic):
            # per-chunk xT tile + x row loads; transposes come as parts
            xt_sb = xt_p.tile([128, KT, 512], MMDT, tag="xt")
            xrs = []
            for k4, it in enumerate(range(4 * ic, 4 * ic + 4)):
                xr = xrow_p.tile([128, E], F32R, tag="xr", bufs=8)
                nc.sync.dma_start(
                    out=xr[:], in_=x[it * 128:(it + 1) * 128, :].bitcast(F32R))
                xrs.append(xr)
            return xt_sb, xrs

        def stage_a_part(xt_sb, xrs, part):
            kt = part
            pt = psum_t.tile([128, 512], F32, tag="pt")
            for k4 in range(4):
                nc.tensor.transpose(
                    pt[:, k4 * 128:(k4 + 1) * 128].bitcast(F32R),
                    xrs[k4][:, kt * 128:(kt + 1) * 128],
                    ident_sb[:])
            nc.scalar.copy(out=xt_sb[:, kt, :], in_=pt[:])

        def stage_a(ic):
            xt_sb, xrs = stage_a_alloc(ic)
            for part in range(KT):
                stage_a_part(xt_sb, xrs, part)
            return xt_sb

        def stage_b(ic, xt_sb):
            for (w_sb, b_sb, o_sb, osl) in (
                    (wq_sb, bq_sb, qt_sb, slice(ic * 512, (ic + 1) * 512)),
                    (wk_sb, bk_sb, kt_sb, slice(ic * 512, (ic + 1) * 512))):
                for nt in range(2):
                    pm = psum_mm.tile([128, 512], F32, tag="pqk")
                    for kt in range(KT):
                        nc.tensor.matmul(
                            pm[:],
                            w_sb[:, kt, nt * 128:(nt + 1) * 128],
                            xt_sb[:, kt, :],
                            start=(kt == 0), stop=(kt == KT - 1),
                        )
                    nc.vector.tensor_scalar_add(
                        out=o_sb[:, nt, osl],
                        in0=pm[:], scalar1=b_sb[:, nt:nt + 1])
            for k4, it in enumerate(range(4 * ic, 4 * ic + 4)):
                pv = psum_mm.tile([128, NSL], F32, tag="pqk")
                for kt in range(KT):
                    nc.tensor.matmul(
                        pv[:],
                        xt_sb[:, kt, k4 * 128:(k4 + 1) * 128],
                        wv_sb[:, kt, :],
                        start=(kt == 0), stop=(kt == KT - 1),
                    )
                nc.vector.tensor_add(
                    out=v_sb[:, it, :, 0:D],
                    in0=pv[:].rearrange("p (h d) -> p h d", d=D),
                    in1=bvb[:].rearrange("p (h d) -> p h d", d=D))

        def stage_c(ic, fillers=None):
            i0 = ic * 512
            njt = 4 * (ic + 1)

            def emit_s(h, jt):
                # diagonal j-tiles only need columns i_local >= 128*dt_
                nt, base = divmod(h, 2)
                base *= D
                dt_ = jt - 4 * ic
                c0 = max(dt_, 0) * 128
                ps = psum_s.tile([128, 512], F32, tag="ps")
                nc.tensor.matmul(
                    ps[:, c0:],
                    kt_sb[base:base + D, nt, jt * 128:(jt + 1) * 128],
                    qt_sb[base:base + D, nt, i0 + c0:i0 + 512],
                    start=True, stop=True,
                )
                pw = pwork.tile([128, 512], MMDT, tag="pw")
                nc.scalar.activation(
                    out=pw[:, c0:], in_=ps[:, c0:],
                    func=mybir.ActivationFunctionType.Exp, scale=float(SCALE))
                if dt_ >= 0:
                    nc.vector.tensor_mul(
                        pw[:, c0:c0 + 128], pw[:, c0:c0 + 128],
                        mask_sb[:, 0, 0:128])
                return pw

            def emit_ctx(h, jt, pc, pw):
                c0 = max(jt - 4 * ic, 0) * 128
                nc.tensor.matmul(
                    pc[:, c0:],
                    v_sb[:, jt, h, :],
                    pw[:, c0:],
                    start=(jt == 0), stop=(jt == njt - 1),
                )

            for h in range(HLOC):
                nt, base = divmod(h, 2)
                base *= D
                pc = psum_c.tile([D + 1, 512], F32, tag="pc")
                pw_prev = emit_s(h, 0)
                for jt in range(1, njt):
                    pw = emit_s(h, jt)
                    emit_ctx(h, jt - 1, pc, pw_prev)
                    pw_prev = pw
                emit_ctx(h, njt - 1, pc, pw_prev)
                lrow = norm_p.tile([1, 512], F32, tag="lrow")
                nc.vector.reciprocal(out=lrow[:], in_=pc[D:D + 1, :])
                lb = norm_p.tile([D, 512], F32, tag="lb")
                nc.gpsimd.partition_broadcast(out_ap=lb[:], in_ap=lrow[:])
                nc.vector.tensor_mul(
                    ctxt_sb[base:base + D, nt, i0:i0 + 512],
                    pc[0:D, :], lb[:])
                for f in (fillers or {}).get(h, []):
                    f()

        def stage_d(ic):
            i0 = ic * 512
            nc.sync.dma_start(
                out=cc_in[ic].rearrange("(t p) i -> p t i", p=128),
                in_=ctxt_sb[:, :, i0:i0 + 512])
            nc.gpsimd.collective_compute(
                "AllGather", mybir.AluOpType.bypass,
                replica_groups=REPLICA_GROUPS,
                ins=[cc_in[ic].opt()], outs=[cc_out[ic].opt()],
            )

        cc_in_h = [dram.tile([NSL, 256], MMDT, name=f"cc_in_h{k}") for k in range(2)]
        cc_out_h = [dram.tile([TP, NSL, 256], MMDT, name=f"cc_out_h{k}")
                    for k in range(2)]

        def stage_d_half(ic, half):
            i0 = ic * 512 + half * 256
            nc.sync.dma_start(
                out=cc_in_h[half].rearrange("(t p) i -> p t i", p=128),
                in_=ctxt_sb[:, :, i0:i0 + 256])
            nc.gpsimd.collective_compute(
                "AllGather", mybir.AluOpType.bypass,
                replica_groups=REPLICA_GROUPS,
                ins=[cc_in_h[half].opt()], outs=[cc_out_h[half].opt()],
            )

        def stage_e_half(ic, half):
            ctxf_sb = ctxf_p.tile([128, KT, 256], MMDT, tag="ctxf_h", bufs=2)
            nc.sync.dma_start(
                out=ctxf_sb[:],
                in_=cc_out_h[half].rearrange("g (t p) i -> p (g t) i", p=128))
            for k in range(2):
                it = 4 * ic + 2 * half + k
                po = psum_mm.tile([128, NSL], F32, tag="pqk")
                for ct in range(KT):
                    nc.tensor.matmul(
                        po[:],
                        ctxf_sb[:, ct, k * 128:(k + 1) * 128],
                        wo_sb[:, ct, :],
                        start=(ct == 0), stop=(ct == KT - 1),
                    )
                ot = osb_p.tile([128, NSL], F32, tag="ot")
                nc.vector.tensor_add(out=ot[:], in0=po[:], in1=bob[:])
                nc.gpsimd.dma_start(out=out[it * 128:(it + 1) * 128, :], in_=ot[:])

        def stage_e_load(ic):
            ctxf_sb = ctxf_p.tile([128, KT, 512], MMDT, tag="ctxf")
            nc.sync.dma_start(
                out=ctxf_sb[:],
                in_=cc_out[ic].rearrange("g (t p) i -> p (g t) i", p=128))
            return ctxf_sb

        def stage_e_part(ic, ctxf_sb, k):
            it = 4 * ic + k
            po = psum_mm.tile([128, NSL], F32, tag="pqk")
            for ct in range(KT):
                nc.tensor.matmul(
                    po[:],
                    ctxf_sb[:, ct, k * 128:(k + 1) * 128],
                    wo_sb[:, ct, :],
                    start=(ct == 0), stop=(ct == KT - 1),
                )
            ot = osb_p.tile([128, NSL], F32, tag="ot")
            nc.vector.tensor_add(out=ot[:], in0=po[:], in1=bob[:])
            nc.gpsimd.dma_start(out=out[it * 128:(it + 1) * 128, :], in_=ot[:])

        def stage_e(ic):
            ctxf_sb = stage_e_load(ic)
            for k in range(4):
                stage_e_part(ic, ctxf_sb, k)

        # pipeline: A(ic+1) transpose parts and E(ic-1) out-proj groups are
        # interleaved between C(ic) attention heads to keep PE dense; the
        # chunked allgather(ic) hides behind C(ic+1).
        emitted = None
        xt_cur = stage_a(0)
        for ic in range(ICH):
            stage_b(ic, xt_cur)
            fillers = {h: [] for h in range(HLOC)}
            if ic + 1 < ICH:
                xt_next, xrs_next = stage_a_alloc(ic + 1)
                for h in range(HLOC):
                    k = 2 * h
                    fillers[h].append(
                        lambda xt=xt_next, xr=xrs_next, kk=k:
                            stage_a_part(xt, xr, kk))
                    fillers[h].append(
                        lambda xt=xt_next, xr=xrs_next, kk=k + 1:
                            stage_a_part(xt, xr, kk))
            else:
                xt_next = None
            if ic > 0:
                ctxf_prev = stage_e_load(ic - 1)
                for h in range(HLOC):
                    fillers[h].append(
                        lambda icc=ic - 1, cf=ctxf_prev, kk=h:
                            stage_e_part(icc, cf, kk))
            stage_c(ic, fillers)
            stage_d(ic)
            xt_cur = xt_next
        stage_e(ICH - 1)


# revision 47
# speedup vs baseline: 1.1062x; 1.0005x over previous
"""Multi-head causal attention (B=2, S=2048, E=1024, H=16, D=64) on 8 trn2 cores.

Sharding (Megatron-style, per hint): data-parallel over batch (2) x
tensor-parallel over heads (4 groups of 4 heads / 256 features).
Core c: batch c//4, head-group c%4.

Per-core device program (SPMD, identical on all cores), pipelined over 4
query chunks of 512:
  A(ic). PE-transpose x rows -> xT (bf16) for the chunk's 4 i-tiles
  B(ic). qT/kT projections in [n, s] layout; v in natural [s, n] layout (bf16)
  C(ic). causal attention in transposed-score layout:
       sT[j,i] = kT_h . qT_h (K=64 matmul), p = exp(s/8) on ScalarE (bf16),
       causal mask via multiplicative 0/1 tiles on diagonal blocks,
       ctxT[d,i] accumulated with v-augmented-with-ones stationary ->
       row 64 of psum = softmax denominator; normalize with
       copy + partition_broadcast + tensor_tensor divide
  D(ic). AllGather the normalized ctxT chunk across the 4-core batch group
  E(ic). out[:, g*256:(g+1)*256] = ctxT_full.T @ Wo[:, slice] + bo[slice]
       (E is emitted one chunk behind so the collective hides behind compute)
Host only slices inputs and concatenates the 8 disjoint output slices.
"""

import contextlib

import ml_dtypes
import numpy as np

import concourse.mybir as mybir
import concourse.tile as tile
from concourse import bacc
from concourse.bass_utils import run_bass_kernel_spmd

F32 = mybir.dt.float32
BF16 = mybir.dt.bfloat16
F32R = mybir.dt.float32r
import os
MMDT = {"bf16": BF16, "f32r": F32R}[os.environ.get("MM_DT", "f32r")]

B, S, E, H, D = 2, 2048, 1024, 16, 64
N_CORES = 8
TP = 4                 # tensor-parallel degree (head groups per batch)
NSL = E // TP          # 256 features per core
HLOC = H // TP         # 4 heads per core
KT = E // 128          # 8 contraction tiles
IT = S // 128          # 16 sequence tiles
ICH = S // 512         # 4 sequence chunks of 512
SCALE = 1.0 / np.sqrt(D)

REPLICA_GROUPS = [[0, 1, 2, 3], [4, 5, 6, 7]]

_cache: dict = {}


def _emit(nc, tc, prm):
    x, wq, bq, wk, bk, wv, bv, wo, bo, ident, masks, out = prm

    with contextlib.ExitStack() as stack:
        ent = stack.enter_context
        const = ent(tc.tile_pool(name="const", bufs=1))
        wstage = ent(tc.tile_pool(name="wstage", bufs=2))
        wpool = ent(tc.tile_pool(name="wpool", bufs=1))
        xrow_p = ent(tc.tile_pool(name="xrow", bufs=2))
        xt_p = ent(tc.tile_pool(name="xt", bufs=2))
        qkv_p = ent(tc.tile_pool(name="qkv", bufs=1))
        psum_t = ent(tc.tile_pool(name="psum_t", bufs=2, space="PSUM"))
        psum_mm = ent(tc.tile_pool(name="psum_mm", bufs=2, space="PSUM"))
        psum_s = ent(tc.tile_pool(name="psum_s", bufs=2, space="PSUM"))
        psum_c = ent(tc.tile_pool(name="psum_c", bufs=2, space="PSUM"))
        pwork = ent(tc.tile_pool(name="pwork", bufs=4))
        norm_p = ent(tc.tile_pool(name="norm", bufs=2))
        ctxt_p = ent(tc.tile_pool(name="ctxt", bufs=1))
        ctxf_p = ent(tc.tile_pool(name="ctxf", bufs=1))
        osb_p = ent(tc.tile_pool(name="osb", bufs=2))
        dram = ent(tc.tile_pool(name="dram", bufs=1, space="DRAM"))

        # ---- constants ----
        ident_sb = const.tile([128, 128], F32R)
        nc.sync.dma_start(out=ident_sb[:], in_=ident[:].bitcast(F32R))
        emitted = {}
        warm_in = dram.tile([1, 128], F32)
        warm_out = dram.tile([TP, 1, 128], F32)
        nc.gpsimd.collective_compute(
            "AllGather", mybir.AluOpType.bypass,
            replica_groups=REPLICA_GROUPS,
            ins=[warm_in.opt()], outs=[warm_out.opt()],
        )
        mask_sb = const.tile([128, HLOC, 512], BF16)

        # ---- weights (dtype MMDT; f32r loads directly, bf16 converts) ----
        wq_sb = wpool.tile([128, KT, NSL], MMDT)
        wk_sb = wpool.tile([128, KT, NSL], MMDT)
        wv_sb = wpool.tile([128, KT, NSL], MMDT)
        wo_sb = wpool.tile([128, KT, NSL], MMDT)
        for w_sb, w_dr in ((wq_sb, wq), (wk_sb, wk), (wv_sb, wv), (wo_sb, wo)):
            if MMDT == F32R:
                w_r = w_dr.rearrange("(t p) n -> p t n", p=128).bitcast(F32R)
                for kt in range(KT):
                    nc.gpsimd.dma_start(
                        out=w_sb[:, kt, :], in_=w_r[:, kt, :])
            else:
                wst = wstage.tile([128, KT, NSL], F32, tag="wst")
                nc.gpsimd.dma_start(
                    out=wst[:], in_=w_dr.rearrange("(t p) n -> p t n", p=128))
                nc.vector.tensor_copy(w_sb[:], wst[:])
        nc.gpsimd.dma_start(out=mask_sb[:], in_=masks[:])
        bq_sb = wpool.tile([128, 2], F32)
        bk_sb = wpool.tile([128, 2], F32)
        for b_sb, b_dr in ((bq_sb, bq), (bk_sb, bk)):
            nc.sync.dma_start(out=b_sb[:], in_=b_dr.rearrange("(t p) -> p t", p=128))
        bv_row = wpool.tile([1, NSL], F32)
        nc.sync.dma_start(out=bv_row[:], in_=bv[None, :])
        bvb = wpool.tile([128, NSL], F32)
        nc.gpsimd.partition_broadcast(out_ap=bvb[:], in_ap=bv_row[:])
        bo_row = wpool.tile([1, NSL], F32)
        nc.sync.dma_start(out=bo_row[:], in_=bo[None, :])
        bob = wpool.tile([128, NSL], F32)
        nc.gpsimd.partition_broadcast(out_ap=bob[:], in_ap=bo_row[:])

        # ---- persistent activations ----
        qt_sb = qkv_p.tile([128, 2, S], MMDT)
        kt_sb = qkv_p.tile([128, 2, S], MMDT)
        v_sb = qkv_p.tile([128, IT, HLOC, D + 1], MMDT)
        ones_col = qkv_p.tile([128, IT, HLOC, 1], F32)
        nc.vector.memset(ones_col[:], 1.0)
        nc.vector.tensor_copy(v_sb[:, :, :, D:D + 1], ones_col[:])
        ctxt_sb = ctxt_p.tile([128, 2, S], MMDT)

        # DRAM bounce buffers for the chunked allgather (distinct per chunk
        # so chunk ic+1's send never waits on chunk ic's collective)
        cc_in = [dram.tile([NSL, 512], MMDT, name=f"cc_in{ic}") for ic in range(ICH)]
        cc_out = [dram.tile([TP, NSL, 512], MMDT, name=f"cc_out{ic}")
                  for ic in range(ICH)]

        def stage_a_alloc(ic):
            # per-chunk xT tile + x row loads; transposes come as parts
            xt_sb = xt_p.tile([128, KT, 512], MMDT, tag="xt")
            xrs = []
            for k4, it in enumerate(range(4 * ic, 4 * ic + 4)):
                xr = xrow_p.tile([128, E], F32R, tag="xr", bufs=8)
                nc.sync.dma_start(
                    out=xr[:], in_=x[it * 128:(it + 1) * 128, :].bitcast(F32R))
                xrs.append(xr)
            return xt_sb, xrs

        def stage_a_part(xt_sb, xrs, part):
            kt = part
            pt = psum_t.tile([128, 512], F32, tag="pt")
            for k4 in range(4):
                nc.tensor.transpose(
                    pt[:, k4 * 128:(k4 + 1) * 128].bitcast(F32R),
                    xrs[k4][:, kt * 128:(kt + 1) * 128],
                    ident_sb[:])
            nc.scalar.copy(out=xt_sb[:, kt, :], in_=pt[:])

        def stage_a(ic):
            xt_sb, xrs = stage_a_alloc(ic)
            for part in range(KT):
                stage_a_part(xt_sb, xrs, part)
            return xt_sb

        def stage_b(ic, xt_sb):
            for (w_sb, b_sb, o_sb, osl) in (
                    (wq_sb, bq_sb, qt_sb, slice(ic * 512, (ic + 1) * 512)),
                    (wk_sb, bk_sb, kt_sb, slice(ic * 512, (ic + 1) * 512))):
                for nt in range(2):
                    pm = psum_mm.tile([128, 512], F32, tag="pqk")
                    for kt in range(KT):
                        nc.tensor.matmul(
                            pm[:],
                            w_sb[:, kt, nt * 128:(nt + 1) * 128],
                            xt_sb[:, kt, :],
                            start=(kt == 0), stop=(kt == KT - 1),
                        )
                    nc.vector.tensor_scalar_add(
                        out=o_sb[:, nt, osl],
                        in0=pm[:], scalar1=b_sb[:, nt:nt + 1])
            for k4, it in enumerate(range(4 * ic, 4 * ic + 4)):
                pv = psum_mm.tile([128, NSL], F32, tag="pqk")
                for kt in range(KT):
                    nc.tensor.matmul(
                        pv[:],
                        xt_sb[:, kt, k4 * 128:(k4 + 1) * 128],
                        wv_sb[:, kt, :],
                        start=(kt == 0), stop=(kt == KT - 1),
                    )
                nc.vector.tensor_add(
                    out=v_sb[:, it, :, 0:D],
                    in0=pv[:].rearrange("p (h d) -> p h d", d=D),
                    in1=bvb[:].rearrange("p (h d) -> p h d", d=D))

        def stage_c(ic, fillers=None):
            i0 = ic * 512
            njt = 4 * (ic + 1)

            def emit_s(h, jt):
                # diagonal j-tiles only need columns i_local >= 128*dt_
                nt, base = divmod(h, 2)
                base *= D
                dt_ = jt - 4 * ic
                c0 = max(dt_, 0) * 128
                ps = psum_s.tile([128, 512], F32, tag="ps")
                nc.tensor.matmul(
                    ps[:, c0:],
                    kt_sb[base:base + D, nt, jt * 128:(jt + 1) * 128],
                    qt_sb[base:base + D, nt, i0 + c0:i0 + 512],
                    start=True, stop=True,
                )
                pw = pwork.tile([128, 512], MMDT, tag="pw")
                nc.scalar.activation(
                    out=pw[:, c0:], in_=ps[:, c0:],
                    func=mybir.ActivationFunctionType.Exp, scale=float(SCALE))
                if dt_ >= 0:
                    nc.vector.tensor_mul(
                        pw[:, c0:c0 + 128], pw[:, c0:c0 + 128],
                        mask_sb[:, 0, 0:128])
                return pw

            def emit_ctx(h, jt, pc, pw):
                c0 = max(jt - 4 * ic, 0) * 128
                nc.tensor.matmul(
                    pc[:, c0:],
                    v_sb[:, jt, h, :],
                    pw[:, c0:],
                    start=(jt == 0), stop=(jt == njt - 1),
                )

            for h in range(HLOC):
                nt, base = divmod(h, 2)
                base *= D
                pc = psum_c.tile([D + 1, 512], F32, tag="pc")
                pw_prev = emit_s(h, 0)
                for jt in range(1, njt):
                    pw = emit_s(h, jt)
                    emit_ctx(h, jt - 1, pc, pw_prev)
                    pw_prev = pw
                emit_ctx(h, njt - 1, pc, pw_prev)
                lrow = norm_p.tile([1, 512], F32, tag="lrow")
                nc.vector.reciprocal(out=lrow[:], in_=pc[D:D + 1, :])
                lb = norm_p.tile([D, 512], F32, tag="lb")
                nc.gpsimd.partition_broadcast(out_ap=lb[:], in_ap=lrow[:])
                nc.vector.tensor_mul(
                    ctxt_sb[base:base + D, nt, i0:i0 + 512],
                    pc[0:D, :], lb[:])
                row = 128 * nt + base
                nc.sync.dma_start(
                    out=cc_in[ic][row:row + D, :],
                    in_=ctxt_sb[base:base + D, nt, i0:i0 + 512])
                for f in (fillers or {}).get(h, []):
                    f()

        def stage_d(ic):
            nc.gpsimd.collective_compute(
                "AllGather", mybir.AluOpType.bypass,
                replica_groups=REPLICA_GROUPS,
                ins=[cc_in[ic].opt()], outs=[cc_out[ic].opt()],
            )

        cc_in_h = [dram.tile([NSL, 256], MMDT, name=f"cc_in_h{k}") for k in range(2)]
        cc_out_h = [dram.tile([TP, NSL, 256], MMDT, name=f"cc_out_h{k}")
                    for k in range(2)]

        def stage_d_half(ic, half):
            i0 = ic * 512 + half * 256
            nc.sync.dma_start(
                out=cc_in_h[half].rearrange("(t p) i -> p t i", p=128),
                in_=ctxt_sb[:, :, i0:i0 + 256])
            nc.gpsimd.collective_compute(
                "AllGather", mybir.AluOpType.bypass,
                replica_groups=REPLICA_GROUPS,
                ins=[cc_in_h[half].opt()], outs=[cc_out_h[half].opt()],
            )

        def stage_e_half(ic, half):
            ctxf_sb = ctxf_p.tile([128, KT, 256], MMDT, tag="ctxf_h", bufs=2)
            nc.sync.dma_start(
                out=ctxf_sb[:],
                in_=cc_out_h[half].rearrange("g (t p) i -> p (g t) i", p=128))
            for k in range(2):
                it = 4 * ic + 2 * half + k
                po = psum_mm.tile([128, NSL], F32, tag="pqk")
                for ct in range(KT):
                    nc.tensor.matmul(
                        po[:],
                        ctxf_sb[:, ct, k * 128:(k + 1) * 128],
                        wo_sb[:, ct, :],
                        start=(ct == 0), stop=(ct == KT - 1),
                    )
                ot = osb_p.tile([128, NSL], F32, tag="ot")
                nc.vector.tensor_add(out=ot[:], in0=po[:], in1=bob[:])
                nc.gpsimd.dma_start(out=out[it * 128:(it + 1) * 128, :], in_=ot[:])

        def stage_e_load(ic, split=False):
            ctxf_sb = ctxf_p.tile([128, KT, 512], MMDT, tag="ctxf")
            cc_r = cc_out[ic].rearrange("g (t p) i -> p (g t) i", p=128)
            if split:
                for ct in range(KT):
                    nc.sync.dma_start(out=ctxf_sb[:, ct, :], in_=cc_r[:, ct, :])
            else:
                nc.sync.dma_start(out=ctxf_sb[:], in_=cc_r)
            return ctxf_sb

        def stage_e_part(ic, ctxf_sb, k):
            it = 4 * ic + k
            po = psum_mm.tile([128, NSL], F32, tag="pqk")
            for ct in range(KT):
                nc.tensor.matmul(
                    po[:],
                    ctxf_sb[:, ct, k * 128:(k + 1) * 128],
                    wo_sb[:, ct, :],
                    start=(ct == 0), stop=(ct == KT - 1),
                )
            ot = osb_p.tile([128, NSL], F32, tag="ot")
            nc.vector.tensor_add(out=ot[:], in0=po[:], in1=bob[:])
            nc.gpsimd.dma_start(out=out[it * 128:(it + 1) * 128, :], in_=ot[:])

        def stage_e(ic):
            ctxf_sb = stage_e_load(ic, split=True)
            for k in range(4):
                stage_e_part(ic, ctxf_sb, k)

        # pipeline: A(ic+1) transpose parts and E(ic-1) out-proj groups are
        # interleaved between C(ic) attention heads to keep PE dense; the
        # chunked allgather(ic) hides behind C(ic+1).
        emitted = None
        xt_cur = stage_a(0)
        for ic in range(ICH):
            stage_b(ic, xt_cur)
            fillers = {h: [] for h in range(HLOC)}
            if ic + 1 < ICH:
                xt_next, xrs_next = stage_a_alloc(ic + 1)
                for h in range(HLOC):
                    k = 2 * h
                    fillers[h].append(
                        lambda xt=xt_next, xr=xrs_next, kk=k:
                            stage_a_part(xt, xr, kk))
                    fillers[h].append(
                        lambda xt=xt_next, xr=xrs_next, kk=k + 1:
                            stage_a_part(xt, xr, kk))
            else:
                xt_next = None
            if ic > 0:
                ctxf_prev = stage_e_load(ic - 1)
                for h in range(HLOC):
                    fillers[h].append(
                        lambda icc=ic - 1, cf=ctxf_prev, kk=h:
                            stage_e_part(icc, cf, kk))
            stage_c(ic, fillers)
            stage_d(ic)
            xt_cur = xt_next
        stage_e(ICH - 1)
